# revision 22
# baseline (speedup 1.0000x reference)
"""Trainium2 Bass kernel for nn_Attention_86646670230179 (eager MHA, f32 I/O).

Strategy (8 NeuronCores, tensor-parallel over heads, collective-free):
  - Each core owns 2 of the 16 heads (a 128-row slice of the internal dim).
  - The scalar engine is the critical resource (128 exp ACTIVATEs of
    [128,1024], ~1.11us each = 143us).  This schedule keeps it doing ONLY
    exp: staging DMA issues live on the gpsimd ring, out-proj casts on
    vector, and every projection (both batches) plus the out-projection
    runs as PE "filler" work threaded between the scores/PV matmuls of the
    scalar-bound st loop.
  - Staging DMAs are 512-row chunks in need-order (k of batch 0 first,
    v/q interleaved by first-use) so block (0,0) is DMA-gated as little
    as possible.  Host stages q/k/v as [128, B, 2, KT, 1024] bf16.
  - Per st: scores^T for both heads as a tile_position-packed concurrent
    pair into one f32 PSUM tile; exp on ScalarE (no max subtraction:
    scores ~ N(0,1), scale 1/8 folded into Wq); PV with an appended
    ones-column (unnormalized out + row sums in one accumulation).
  - Normalization: one PSUM->SBUF copy per head (releases the PV bank
    within ~0.7us so 2 PV banks suffice), reciprocal_approx_fast, gpsimd
    partition-broadcast, one vector multiply per head.
  - PSUM banks: scores 2x[128,1024] (4) + PV 2x[128,512] (2) + proj chain
    (1) + out-proj (1) = 8.
  - Host sums the 8 bf16 partial outputs (the TP all-reduce) and adds
    (bv @ Wo + bo), which commutes with attention since softmax rows sum
    to 1.  fp8 was evaluated and rejected: attention-weight quantization
    gives ~3-5% output error, over the 2e-2 gate.
"""
import sys
from contextlib import ExitStack

import numpy as np

sys.path.insert(0, "/opt/trn_rl_repo")

import ml_dtypes  # noqa: E402
import concourse.bass as bass  # noqa: E402
import concourse.mybir as mybir  # noqa: E402
import concourse.tile as tile  # noqa: E402
from concourse import bacc  # noqa: E402
from concourse.bass_utils import run_bass_kernel_spmd  # noqa: E402
from concourse.masks import make_identity  # noqa: E402

BF16 = mybir.dt.bfloat16
F32 = mybir.dt.float32
AF = mybir.ActivationFunctionType

NCORES = 8
B, L, E, H = 2, 2048, 1024, 16
S = L
D = E // H            # 64 head dim
R = B * L             # 4096 total rows
HC = H // NCORES      # 2 heads per core
EC = HC * D           # 128 channel slice per core
KT = E // 128         # 8 contraction tiles
NT = L // 512         # 4 512-wide row tiles per batch
NP = L // 1024        # 2 1024-wide projection pairs per batch
ST = S // 128         # 16 key tiles per batch
STN = ST // NT        # 4 key tiles per 512-row block
DP1 = D + 1           # 65: head dim + ones column


def build_nc():
    nc = bacc.Bacc("TRN2", target_bir_lowering=False, num_devices=NCORES)

    qT = nc.declare_dram_parameter("qT", [128, B, 2, KT, 1024], BF16,
                                   isOutput=False)
    kT = nc.declare_dram_parameter("kT", [128, B, 2, KT, 1024], BF16,
                                   isOutput=False)
    vT = nc.declare_dram_parameter("vT", [128, B, 2, KT, 1024], BF16,
                                   isOutput=False)
    wq = nc.declare_dram_parameter("wq", [128, KT * EC], BF16, isOutput=False)
    wk = nc.declare_dram_parameter("wk", [128, KT * EC], BF16, isOutput=False)
    wv = nc.declare_dram_parameter("wv", [128, KT * EC], BF16, isOutput=False)
    wo = nc.declare_dram_parameter("wo", [128, E], BF16, isOutput=False)
    bq = nc.declare_dram_parameter("bq", [EC, 1], F32, isOutput=False)
    bk = nc.declare_dram_parameter("bk", [EC, 1], F32, isOutput=False)
    outTp = nc.declare_dram_parameter("outTp", [E, R], BF16, isOutput=True)

    with tile.TileContext(nc) as tc, ExitStack() as ctx:
        consts = ctx.enter_context(tc.tile_pool(name="consts", bufs=1))
        xt_pool = ctx.enter_context(tc.tile_pool(name="xt", bufs=1))
        vpt_pool = ctx.enter_context(tc.tile_pool(name="vpt", bufs=2))
        exp_pool = ctx.enter_context(tc.tile_pool(name="expp", bufs=12))
        otr_pool = ctx.enter_context(tc.tile_pool(name="otr", bufs=4))
        ot_pool = ctx.enter_context(tc.tile_pool(name="otp", bufs=2))
        ov_pool = ctx.enter_context(tc.tile_pool(name="ovp", bufs=4))
        rc_pool = ctx.enter_context(tc.tile_pool(name="rcp", bufs=4))
        rcb_pool = ctx.enter_context(tc.tile_pool(name="rcbp", bufs=4))
        # PSUM: sc 2x[128,1024] (4 banks) + pv 2 + proj chain 1 + outproj 1
        psum_sc = ctx.enter_context(tc.tile_pool(name="psc", bufs=2, space="PSUM"))
        psum_pv = ctx.enter_context(tc.tile_pool(name="ppv", bufs=2, space="PSUM"))
        psum_pp = ctx.enter_context(tc.tile_pool(name="ppp", bufs=1, space="PSUM"))
        psum_oj = ctx.enter_context(tc.tile_pool(name="poj", bufs=1, space="PSUM"))

        # ---- weights + biases on the sync ring (small, land ~3us)
        wq_sb = consts.tile([128, KT, EC], BF16, tag="wq")
        wk_sb = consts.tile([128, KT, EC], BF16, tag="wk")
        wv_sb = consts.tile([128, KT, EC], BF16, tag="wv")
        wo_sb = consts.tile([128, KT, EC], BF16, tag="wo")
        # wk/wq + biases ride the gpsimd ring ahead of the k chunks (it
        # starts fastest; the sync ring's first transfer can lag ~10us) --
        # they gate the first projections and their bias epilogues
        nc.gpsimd.dma_start(wk_sb[:], wk[:].rearrange("p (ko m) -> p ko m", m=EC))
        nc.gpsimd.dma_start(wq_sb[:], wq[:].rearrange("p (ko m) -> p ko m", m=EC))
        bq_sb = consts.tile([EC, 1], F32, tag="bq")
        bk_sb = consts.tile([EC, 1], F32, tag="bk")
        nc.gpsimd.dma_start(bq_sb[:], bq[:])
        nc.gpsimd.dma_start(bk_sb[:], bk[:])
        nc.gpsimd.dma_start(wv_sb[:], wv[:].rearrange("p (ko m) -> p ko m", m=EC))
        ident = consts.tile([128, 128], BF16, tag="ident")
        make_identity(nc, ident[:])
        ones_row = consts.tile([1, D], F32, tag="ones")
        nc.vector.memset(ones_row[:], 1.0)

        # ---- staged activations: one [128, KT, L] buffer per tensor,
        # shared across batches (batch 1 overwrites once batch 0 is
        # consumed); filled in 512-row chunks on the gpsimd ring.
        staged = {}
        for name in ("k", "v", "q"):
            staged[name] = xt_pool.tile([128, KT, L], BF16, tag=f"xt{name}",
                                        name=f"xt{name}")
        xsrc = {"k": kT, "v": vT, "q": qT}

        rings = {"k": nc.gpsimd, "q": nc.sync, "v": nc.sync}

        def stage_chunk(name, b, c0, c1, eng=None):
            if eng == "scalar":
                eng = nc.scalar
            """rows [512*c0, 512*c1) of batch b for tensor `name`.  k gets
            its own ring (gpsimd); v/q share sync; the two pre-exp q chunks
            ride the scalar ring while it is still idle, so three DMA
            queues pull concurrently during the critical first block."""
            for h in range(2):
                r0 = max(c0 * 512, h * 1024)
                r1 = min(c1 * 512, (h + 1) * 1024)
                if r0 >= r1:
                    continue
                (eng or rings[name]).dma_start(
                    staged[name][:, :, r0:r1],
                    xsrc[name][:, b, h, :, r0 - h * 1024:r1 - h * 1024],
                )

        # projected activations (persistent, per batch)
        qpT = [[consts.tile([128, 1024], BF16, tag=f"qpT{b}_{p}",
                            name=f"qpT{b}_{p}") for p in range(NP)]
               for b in range(B)]
        kpT = [[consts.tile([128, 1024], BF16, tag=f"kpT{b}_{p}",
                            name=f"kpT{b}_{p}") for p in range(NP)]
               for b in range(B)]
        vp = [[consts.tile([128, STN, 2 * DP1], BF16, tag=f"vp{b}_{n}",
                           name=f"vp{b}_{n}")
               for n in range(NT)] for b in range(B)]
        for b in range(B):
            for n in range(NT):
                nc.vector.memset(vp[b][n][:, :, D], 1.0)
                nc.vector.memset(vp[b][n][:, :, 2 * D + 1], 1.0)

        # ---- HAM warmup: real matmul activity spanning the first DMA
        # wait so projections run at 2.4GHz, not the cold 1.2GHz.  The
        # later waves use the weight tiles as rhs so they pace themselves
        # behind the weight DMAs.  (transpose-mode does not warm HAM.)
        # single accumulation group: back-to-back streaming, no per-matmul
        # semaphore round-trips (separate tiles would WAW-serialize)
        wps = psum_pp.tile([128, 128], F32, tag="pp", name="warm")
        for j in range(24):
            nc.tensor.matmul(wps[:], lhsT=ident[:], rhs=ident[:],
                             start=(j == 0), stop=False)
        for kt in range(KT):
            for j in range(3):
                nc.tensor.matmul(wps[:], lhsT=ident[:], rhs=wk_sb[:, kt, :],
                                 start=False,
                                 stop=(kt == KT - 1 and j == 2))

        # ---------- filler units ----------
        # A unit is a list of (pe_cost, closure) ops.  Units are kept in a
        # FIFO; ops are popped a few per st (budget), gated on a DMA-
        # readiness st (gate) and force-drained at the start of the block
        # that consumes their output (need) so a consumer is never emitted
        # before its producer (deadlock-proof).

        def proj_chain_ops(b, name, p, hf):
            """8 matmuls + bias epilogue producing kpT/qpT[b][p] half hf."""
            w_sb, bias, dst = {
                "k": (wk_sb, bk_sb, kpT),
                "q": (wq_sb, bq_sb, qpT),
            }[name]
            box = {}
            ops = []
            for kt in range(KT):
                def mm(kt=kt):
                    if kt == 0:
                        box["ps"] = psum_pp.tile([128, 512], F32, tag="pp",
                                                 name="prch")
                    nc.tensor.matmul(
                        box["ps"][:], lhsT=w_sb[:, kt, :],
                        rhs=staged[name][:, kt,
                                         p * 1024 + hf * 512:
                                         p * 1024 + (hf + 1) * 512],
                        start=(kt == 0), stop=(kt == KT - 1),
                    )
                ops.append((1.0, mm))

            def epi():
                nc.vector.tensor_tensor(
                    dst[b][p][:, hf * 512:(hf + 1) * 512], box["ps"][:],
                    bias[:].to_broadcast((EC, 512)), mybir.AluOpType.add,
                )
            ops.append((0.1, epi))
            return ops

        def vproj_chain_ops(b, p, hf, alt=False):
            """vp[b][2p+hf] projected DIRECTLY in [key, dim] layout: the
            staged activation is the stationary operand, so out = v.T @ Wv
            per 128-key slice -- no on-chip transpose, no cast ladder.
            One strided copy per slice drops the result into the two
            65-wide head slots (ones column preserved)."""
            n = p * 2 + hf
            ops = []
            for sl in range(STN):
                box = {}
                base = p * 1024 + hf * 512 + sl * 128
                pool = psum_oj if (alt and sl % 2 == 1) else psum_pp
                for kt in range(KT):
                    def mm(kt=kt, sl=sl, box=box, base=base, pool=pool):
                        if kt == 0:
                            box["ps"] = pool.tile(
                                [128, 128], F32,
                                tag=("oj" if pool is psum_oj else "pp"),
                                name="vch")
                        nc.tensor.matmul(
                            box["ps"][:],
                            lhsT=staged["v"][:, kt, base:base + 128],
                            rhs=wv_sb[:, kt, :],
                            start=(kt == 0), stop=(kt == KT - 1),
                        )
                    ops.append((0.45, mm))

                def cp(sl=sl, box=box):
                    nc.vector.tensor_copy(
                        vp[b][n][:, sl, :].rearrange(
                            "p (h d) -> p h d", h=2)[:, :, 0:D],
                        box["ps"][:].rearrange("p (h d) -> p h d", h=2),
                    )
                ops.append((0.1, cp))
            return ops

        fillers = []  # FIFO of {gate, need, ops: [(cost, op), ...]}

        def add_unit(gate, need, ops):
            fillers.append({"gate": gate, "need": need, "ops": list(ops)})

        ojq = []  # [(gate, op)] out-projection units, 1 popped per st

        def oj_unit(ot, rowbase, m, pool=None, scalar_cast=False):
            def op():
                # once the filler chains have drained, alternate the ojs
                # across two PSUM banks so the matmul never waits the
                # previous oj's vector cast
                pl = pool
                if pl is None and not fillers and m % 2 == 1:
                    pl = psum_pp
                pt = (pl or psum_oj).tile([128, 512], F32,
                                          tag=("pp" if pl is psum_pp else "oj"),
                                          name="ojp")
                nc.tensor.matmul(pt[:], lhsT=wo_sb[:, m, :], rhs=ot[:],
                                 start=True, stop=True)
                ov = ov_pool.tile([128, 512], BF16, tag="ov", name="ovt")
                if scalar_cast:
                    nc.scalar.copy(ov[:], pt[:])
                else:
                    nc.vector.tensor_copy(ov[:], pt[:])
                nc.sync.dma_start(
                    outTp[m * 128:(m + 1) * 128, rowbase:rowbase + 512], ov[:]
                )
            return op

        def pop_fillers(g, budget=2.4):
            if ojq and g >= ojq[0][0] and g % 2 == 1:
                ojq.pop(0)[1]()
                budget -= 1.0
            spent = 0.0
            while fillers and spent < budget:
                u = fillers[0]
                if g < u["gate"] and u["need"] > g + 8:
                    break
                cost, op = u["ops"].pop(0)
                op()
                spent += cost
                if not u["ops"]:
                    fillers.pop(0)

        def force_units(max_need):
            """Emit every queued unit needed by st <= max_need (and, by
            FIFO, everything ahead of it)."""
            last = -1
            for i, u in enumerate(fillers):
                if u["need"] <= max_need:
                    last = i
            for u in fillers[:last + 1]:
                for _, op in u["ops"]:
                    op()
            del fillers[:last + 1]

        # ---------- core attention ops ----------

        def scores_exp(b, lt, st):
            ps = psum_sc.tile([128, 1024], F32, tag="sc", name="psc")
            for h in range(HC):
                nc.tensor.matmul(
                    ps[:, h * 512:(h + 1) * 512],
                    lhsT=kpT[b][st // 8][h * D:(h + 1) * D,
                                         (st % 8) * 128:(st % 8 + 1) * 128],
                    rhs=qpT[b][lt // 2][h * D:(h + 1) * D,
                                        (lt % 2) * 512:(lt % 2) * 512 + 512],
                    start=True, stop=True,
                    tile_position=(h * D, 0),
                )
            ex = exp_pool.tile([128, 1024], BF16, tag="exp", name="ext")
            nc.scalar.activation(ex[:], ps[:], AF.Exp)
            return ex

        def pv(b, po, st, ex, first, last):
            for h in range(HC):
                nc.tensor.matmul(
                    po[h][0:DP1, :],
                    lhsT=vp[b][st // STN][:, st % STN, h * DP1:(h + 1) * DP1],
                    rhs=ex[:, h * 512:(h + 1) * 512],
                    start=first, stop=last,
                )

        def norm_and_queue_oj(b, lt, po, g_end, tail=False):
            """Copy PSUM->SBUF (frees PV banks fast), normalize, queue the
            out-projection units (gated 3 sts later so the ot multiply has
            landed before the first oj matmul reaches the PE)."""
            otrs, rcs = [], []
            for h in range(HC):
                otr = otr_pool.tile([D, 512], F32, tag="otr", name="otrt")
                sm = rc_pool.tile([1, 512], F32, tag="sm", name="smt")
                if tail and h == 0:
                    # scalar engine is idle after the last exp: split the
                    # tail normalization across engines
                    nc.scalar.copy(otr[:], po[h][0:D, :])
                    nc.scalar.copy(sm[:], po[h][D:DP1, :])
                else:
                    nc.vector.tensor_copy(otr[:], po[h][0:D, :])
                    nc.vector.tensor_copy(sm[:], po[h][D:DP1, :])
                otrs.append(otr)
                rcs.append(sm)
            for h in range(HC):
                rc1 = rc_pool.tile([1, 512], F32, tag="rc1", name="rc1t")
                nc.vector.reciprocal_approx_fast(rc1[:], rcs[h][:])
                rcs[h] = rc1
            rcbs = []
            for h in range(HC):
                if tail:
                    # PE is idle in the tail: outer-product broadcast beats
                    # the ~1us gpsimd library call on the critical path
                    rcb = psum_sc.tile([D, 512], F32, tag="sc", name="rcbp")
                    nc.tensor.matmul(rcb[:], lhsT=ones_row[:],
                                     rhs=rcs[h][:], start=True, stop=True)
                else:
                    rcb = rcb_pool.tile([D, 512], F32, tag="rcb", name="rcbt")
                    nc.gpsimd.partition_broadcast(rcb[:], rcs[h][:])
                rcbs.append(rcb)
            ot = ot_pool.tile([128, 512], BF16, tag="ot", name="ott")
            for h in range(HC):
                nc.vector.tensor_tensor(
                    ot[h * D:(h + 1) * D, :], otrs[h][:], rcbs[h][:],
                    mybir.AluOpType.mult,
                )
            rowbase = b * L + lt * 512
            for m in range(KT):
                pool = psum_pp if (tail and m % 2 == 1) else None
                ojq.append((g_end + 3, oj_unit(ot, rowbase, m, pool=pool)))

        # ---------- staging schedule (3 rings, need-order) --------------
        stage_chunk("k", 0, 0, 1)
        stage_chunk("q", 0, 0, 1, eng=nc.scalar)
        stage_chunk("q", 0, 1, 2, eng=nc.scalar)
        nc.gpsimd.dma_start(wo_sb[:], wo[:].rearrange("p (m o) -> p m o", o=EC))
        dma_plan = {
            0: [("v", 0, 0, 1), ("k", 0, 1, 2)],
            2: [("v", 0, 1, 2), ("k", 0, 2, 4)],
            4: [("v", 0, 2, 4)],
            6: [("q", 0, 2, 4)],
            16: [("k", 1, 0, 2)], 17: [("k", 1, 2, 4)],
            22: [("v", 1, 0, 2)], 24: [("v", 1, 2, 4)],
            42: [("q", 1, 0, 2)], 44: [("q", 1, 2, 4)],
        }

        # ---------- pre-loop projections (block (0,0) first sts) --------
        for _, op in proj_chain_ops(0, "k", 0, 0):
            op()
        for _, op in proj_chain_ops(0, "q", 0, 0):
            op()

        # ---------- filler unit queue, ordered by need (FIFO) -----------
        # gate = earliest st the staging DMA has landed (pop paced);
        # need = first global st whose scores/PV consumes the output
        # (lookahead-forced so a consumer is never emitted first).
        add_unit(23, 32, proj_chain_ops(0, "q", 1, 0))   # blocks (0,2)+
        add_unit(25, 40, proj_chain_ops(0, "q", 1, 1))
        add_unit(28, 64, proj_chain_ops(1, "k", 0, 0))
        add_unit(39, 64, vproj_chain_ops(1, 0, 0))
        add_unit(44, 64, proj_chain_ops(1, "q", 0, 0))
        add_unit(29, 68, proj_chain_ops(1, "k", 0, 1))
        add_unit(40, 68, vproj_chain_ops(1, 0, 1))
        add_unit(34, 72, proj_chain_ops(1, "k", 1, 0))
        add_unit(49, 72, vproj_chain_ops(1, 1, 0))
        add_unit(35, 76, proj_chain_ops(1, "k", 1, 1))
        add_unit(50, 76, vproj_chain_ops(1, 1, 1))
        add_unit(45, 80, proj_chain_ops(1, "q", 0, 1))
        add_unit(54, 96, proj_chain_ops(1, "q", 1, 0))
        add_unit(55, 112, proj_chain_ops(1, "q", 1, 1))

        # ---------- block (0,0): hand-scheduled (DMA-paced) -------------
        # Interleaves the remaining batch-0 projections between the scores
        # (chains sequential: they share the single pp PSUM bank) with PV
        # emission deferred until each vp quarter's transposes are out.
        vch = {n: vproj_chain_ops(0, n // 2, n % 2, alt=True)
               for n in range(NT)}
        kch = {1: proj_chain_ops(0, "k", 0, 1),
               2: proj_chain_ops(0, "k", 1, 0),
               3: proj_chain_ops(0, "k", 1, 1)}
        qch1 = proj_chain_ops(0, "q", 0, 1)
        # per-st: (list of (chain, lo, hi) to emit, PV sts to emit after)
        b0_plan = {
            1: ([(kch[1], 0, 5)], []),
            2: ([(kch[1], 5, 9)], []),
            3: ([(vch[0], 0, 18)], []),
            4: ([(vch[0], 18, 36)], [0, 1, 2, 3]),
            5: ([(kch[2], 0, 9)], []),
            6: ([(kch[3], 0, 9)], []),
            7: ([(vch[1], 0, 18)], [4, 5]),
            8: ([(vch[1], 18, 36)], [6, 7]),
            9: ([(vch[2], 0, 18)], []),
            10: ([(vch[2], 18, 36)], [8, 9]),
            11: ([(vch[3], 0, 18)], [10, 11]),
            12: ([(vch[3], 18, 36)], [12]),
            13: ([(qch1, 0, 9)], [13]),
            14: ([], [14]),
            15: ([], [15]),
        }
        po = [psum_pv.tile([128, 512], F32, tag="pv", name=f"po{h}")
              for h in range(HC)]
        exs = {}
        for st in range(ST):
            for ch in dma_plan.get(st, []):
                stage_chunk(*ch)
            exs[st] = scores_exp(0, 0, st)
            pre, pvs = b0_plan.get(st, ([], []))
            for ch, o0, o1 in pre:
                for _, op in ch[o0:o1]:
                    op()
            for p_st in pvs:
                pv(0, po, p_st, exs.pop(p_st), first=(p_st == 0),
                   last=(p_st == ST - 1))
        norm_and_queue_oj(0, 0, po, 15)

        # ---------- blocks 1..7: generic scalar-paced loop --------------
        for blk in range(1, 8):
            b, lt = blk // 4, blk % 4
            po = [psum_pv.tile([128, 512], F32, tag="pv", name=f"po{h}")
                  for h in range(HC)]
            for st in range(ST):
                g = blk * 16 + st
                for ch in dma_plan.get(g, []):
                    stage_chunk(*ch)
                ex = scores_exp(b, lt, st)
                pv(b, po, st, ex, first=(st == 0), last=(st == ST - 1))
                pop_fillers(g)
                force_units(g + 2)
            norm_and_queue_oj(b, lt, po, blk * 16 + 15, tail=(blk == 7))

        # ---------- tail flush ----------
        while fillers:
            u = fillers.pop(0)
            for _, op in u["ops"]:
                op()
        while ojq:
            ojq.pop(0)[1]()

    nc.compile()
    return nc


_NC_CACHE = {}


def _get_nc():
    if "nc" not in _NC_CACHE:
        _NC_CACHE["nc"] = build_nc()
    return _NC_CACHE["nc"]


def _prearrange(w):
    # [E, EC] -> [128, KT*EC] partition-major so the device DMA is contiguous
    bf = ml_dtypes.bfloat16
    return np.ascontiguousarray(
        w.reshape(KT, 128, EC).transpose(1, 0, 2).reshape(128, KT * EC)
    ).astype(bf)


def kernel(q, k, v, Wq, bq, Wk, bk, Wv, bv, Wo, bo, _trace=False, _tmpdir=None):
    bf = ml_dtypes.bfloat16
    scale = np.float32(1.0 / np.sqrt(D))  # 0.125, exact

    def _stage_x(x):
        # [B, L, E] -> [128, B, 2, KT, 1024]: partition-major staging with
        # each L-half contiguous per partition (long DMA lines, few issues)
        xt = np.asarray(x, np.float32).reshape(B, 2, 1024, KT, 128)
        return np.ascontiguousarray(xt.transpose(4, 0, 1, 3, 2)).astype(bf)

    qTh = _stage_x(q)
    kTh = _stage_x(k)
    vTh = _stage_x(v)
    Wq = np.asarray(Wq, np.float32)
    Wk = np.asarray(Wk, np.float32)
    Wv = np.asarray(Wv, np.float32)
    Wo = np.asarray(Wo, np.float32)

    in_maps = []
    for c in range(NCORES):
        sl = slice(c * EC, (c + 1) * EC)
        in_maps.append({
            "qT": qTh,
            "kT": kTh,
            "vT": vTh,
            "wq": _prearrange(Wq[:, sl] * scale),
            "wk": _prearrange(Wk[:, sl]),
            "wv": _prearrange(Wv[:, sl]),
            "wo": np.ascontiguousarray(Wo[sl, :]).astype(bf),
            "bq": (np.asarray(bq, np.float32)[sl] * scale).reshape(EC, 1).copy(),
            "bk": np.asarray(bk, np.float32)[sl].reshape(EC, 1).copy(),
        })

    nc = _get_nc()
    res = run_bass_kernel_spmd(
        nc, in_maps, list(range(NCORES)), trace=_trace, tmpdir=_tmpdir
    )
    # sum the per-core partial outputs (the all-reduce of the TP sharding)
    acc = np.zeros((E, R), np.float32)
    for c in range(NCORES):
        acc += np.asarray(res.results[c]["outTp"], np.float32)
    out = np.ascontiguousarray(acc.T)  # [R, E]
    # bv passes through attention unchanged (softmax rows sum to 1):
    # out += bv @ Wo + bo
    host_bias = (
        np.asarray(bv, np.float64) @ np.asarray(Wo, np.float64)
        + np.asarray(bo, np.float64)
    ).astype(np.float32)
    out += host_bias[None, :]
    if _trace:
        return out.reshape(B, L, E), res
    return out.reshape(B, L, E)


# revision 23
# speedup vs baseline: 1.0186x; 1.0186x over previous
"""Trainium2 Bass kernel for nn_Attention_86646670230179 (eager MHA, f32 I/O).

Strategy (8 NeuronCores, tensor-parallel over heads, collective-free):
  - Each core owns 2 of the 16 heads (a 128-row slice of the internal dim).
  - The scalar engine is the critical resource (128 exp ACTIVATEs of
    [128,1024], ~1.11us each = 143us).  The schedule keeps it doing ONLY
    exp: staging DMA issues live on the gpsimd/sync/scalar rings, out-proj
    casts on vector, and every projection (both batches) plus the
    out-projection runs as PE "filler" work threaded between the
    scores/PV matmuls of the scalar-bound st loop (gate = DMA readiness,
    need = consumer deadline; force-emitted before the consuming block so
    a consumer is never queued ahead of its producer).
  - Staging is 512/1024-row chunks in need-order across THREE DMA rings
    (k+critical weights on gpsimd, v/q0c on sync, q pair-0 on the
    otherwise-idle scalar ring) so block (0,0) is gated as little as
    possible; block 0 runs a hand-written DMA-paced schedule.
  - Per st: scores^T for both heads as a tile_position-packed concurrent
    pair into one f32 PSUM tile; exp on ScalarE (no max subtraction:
    scores ~ N(0,1), scale 1/8 folded into Wq); PV with an appended
    ones-column (unnormalized out + row sums in one accumulation).
  - v is projected DIRECTLY into [key, dim] layout (staged activation as
    the stationary operand: vp = v_staged.T @ Wv per 128-key slice) -- no
    on-chip transposes, one strided copy per slice.
  - Normalization per block: PSUM->SBUF copies (release the PV banks in
    <1us so 2 banks suffice), reciprocal_approx_fast, gpsimd partition-
    broadcast, one vector multiply per head.  Out-projections are queued
    and drained one per odd st so the single oj PSUM bank never stalls
    the in-order PE queue.
  - PSUM banks: scores 2x[128,1024] f32 (4) + PV 2 + proj chain 1 +
    out-proj 1 = 8.  A 44-matmul single-accumulation warmup keeps the PE
    HAM clock at 2.4GHz through the first projections.
  - Host sums the 8 bf16 partial outputs (the TP all-reduce) and adds
    (bv @ Wo + bo), which commutes with attention since softmax rows sum
    to 1.  Rejected: fp8 anywhere in the attention-weight path (~3-5%
    output error vs the 2e-2 gate), bf16 matmul PSUM dst (unsupported),
    merged [128,2048] exp (needs 8 PSUM banks for scores alone).
"""
import sys
from contextlib import ExitStack

import numpy as np

sys.path.insert(0, "/opt/trn_rl_repo")

import ml_dtypes  # noqa: E402
import concourse.bass as bass  # noqa: E402
import concourse.mybir as mybir  # noqa: E402
import concourse.tile as tile  # noqa: E402
from concourse import bacc  # noqa: E402
from concourse.bass_utils import run_bass_kernel_spmd  # noqa: E402
from concourse.masks import make_identity  # noqa: E402

BF16 = mybir.dt.bfloat16
F32 = mybir.dt.float32
AF = mybir.ActivationFunctionType

NCORES = 8
B, L, E, H = 2, 2048, 1024, 16
S = L
D = E // H            # 64 head dim
R = B * L             # 4096 total rows
HC = H // NCORES      # 2 heads per core
EC = HC * D           # 128 channel slice per core
KT = E // 128         # 8 contraction tiles
NT = L // 512         # 4 512-wide row tiles per batch
NP = L // 1024        # 2 1024-wide projection pairs per batch
ST = S // 128         # 16 key tiles per batch
STN = ST // NT        # 4 key tiles per 512-row block
DP1 = D + 1           # 65: head dim + ones column


def build_nc():
    nc = bacc.Bacc("TRN2", target_bir_lowering=False, num_devices=NCORES)

    qT = nc.declare_dram_parameter("qT", [128, B, 2, KT, 1024], BF16,
                                   isOutput=False)
    kT = nc.declare_dram_parameter("kT", [128, B, 2, KT, 1024], BF16,
                                   isOutput=False)
    vT = nc.declare_dram_parameter("vT", [128, B, 2, KT, 1024], BF16,
                                   isOutput=False)
    wq = nc.declare_dram_parameter("wq", [128, KT * EC], BF16, isOutput=False)
    wk = nc.declare_dram_parameter("wk", [128, KT * EC], BF16, isOutput=False)
    wv = nc.declare_dram_parameter("wv", [128, KT * EC], BF16, isOutput=False)
    wo = nc.declare_dram_parameter("wo", [128, E], BF16, isOutput=False)
    bq = nc.declare_dram_parameter("bq", [EC, 1], F32, isOutput=False)
    bk = nc.declare_dram_parameter("bk", [EC, 1], F32, isOutput=False)
    outTp = nc.declare_dram_parameter("outTp", [E, R], BF16, isOutput=True)

    with tile.TileContext(nc) as tc, ExitStack() as ctx:
        consts = ctx.enter_context(tc.tile_pool(name="consts", bufs=1))
        xt_pool = ctx.enter_context(tc.tile_pool(name="xt", bufs=1))
        vpt_pool = ctx.enter_context(tc.tile_pool(name="vpt", bufs=2))
        exp_pool = ctx.enter_context(tc.tile_pool(name="expp", bufs=12))
        otr_pool = ctx.enter_context(tc.tile_pool(name="otr", bufs=4))
        ot_pool = ctx.enter_context(tc.tile_pool(name="otp", bufs=2))
        ov_pool = ctx.enter_context(tc.tile_pool(name="ovp", bufs=4))
        rc_pool = ctx.enter_context(tc.tile_pool(name="rcp", bufs=4))
        rcb_pool = ctx.enter_context(tc.tile_pool(name="rcbp", bufs=4))
        # PSUM: sc 2x[128,1024] (4 banks) + pv 2 + proj chain 1 + outproj 1
        psum_sc = ctx.enter_context(tc.tile_pool(name="psc", bufs=2, space="PSUM"))
        psum_pv = ctx.enter_context(tc.tile_pool(name="ppv", bufs=2, space="PSUM"))
        psum_pp = ctx.enter_context(tc.tile_pool(name="ppp", bufs=1, space="PSUM"))
        psum_oj = ctx.enter_context(tc.tile_pool(name="poj", bufs=1, space="PSUM"))

        # ---- weights + biases on the sync ring (small, land ~3us)
        wq_sb = consts.tile([128, KT, EC], BF16, tag="wq")
        wk_sb = consts.tile([128, KT, EC], BF16, tag="wk")
        wv_sb = consts.tile([128, KT, EC], BF16, tag="wv")
        wo_sb = consts.tile([128, KT, EC], BF16, tag="wo")
        # wk/wq + biases ride the gpsimd ring ahead of the k chunks (it
        # starts fastest; the sync ring's first transfer can lag ~10us) --
        # they gate the first projections and their bias epilogues
        nc.gpsimd.dma_start(wk_sb[:], wk[:].rearrange("p (ko m) -> p ko m", m=EC))
        nc.gpsimd.dma_start(wq_sb[:], wq[:].rearrange("p (ko m) -> p ko m", m=EC))
        bq_sb = consts.tile([EC, 1], F32, tag="bq")
        bk_sb = consts.tile([EC, 1], F32, tag="bk")
        nc.gpsimd.dma_start(bq_sb[:], bq[:])
        nc.gpsimd.dma_start(bk_sb[:], bk[:])
        nc.gpsimd.dma_start(wv_sb[:], wv[:].rearrange("p (ko m) -> p ko m", m=EC))
        nc.sync.dma_start(wo_sb[:], wo[:].rearrange("p (m o) -> p m o", o=EC))
        ident = consts.tile([128, 128], BF16, tag="ident")
        make_identity(nc, ident[:])

        # ---- staged activations: one [128, KT, L] buffer per tensor,
        # shared across batches (batch 1 overwrites once batch 0 is
        # consumed); filled in 512-row chunks on the gpsimd ring.
        staged = {}
        for name in ("k", "v", "q"):
            staged[name] = xt_pool.tile([128, KT, L], BF16, tag=f"xt{name}",
                                        name=f"xt{name}")
        xsrc = {"k": kT, "v": vT, "q": qT}

        rings = {"k": nc.gpsimd, "q": nc.sync, "v": nc.sync}

        def stage_chunk(name, b, c0, c1, eng=None):
            if eng == "scalar":
                eng = nc.scalar
            """rows [512*c0, 512*c1) of batch b for tensor `name`.  k gets
            its own ring (gpsimd); v/q share sync; the two pre-exp q chunks
            ride the scalar ring while it is still idle, so three DMA
            queues pull concurrently during the critical first block."""
            for h in range(2):
                r0 = max(c0 * 512, h * 1024)
                r1 = min(c1 * 512, (h + 1) * 1024)
                if r0 >= r1:
                    continue
                (eng or rings[name]).dma_start(
                    staged[name][:, :, r0:r1],
                    xsrc[name][:, b, h, :, r0 - h * 1024:r1 - h * 1024],
                )

        # projected activations (persistent, per batch)
        qpT = [[consts.tile([128, 1024], BF16, tag=f"qpT{b}_{p}",
                            name=f"qpT{b}_{p}") for p in range(NP)]
               for b in range(B)]
        kpT = [[consts.tile([128, 1024], BF16, tag=f"kpT{b}_{p}",
                            name=f"kpT{b}_{p}") for p in range(NP)]
               for b in range(B)]
        vp = [[consts.tile([128, STN, 2 * DP1], BF16, tag=f"vp{b}_{n}",
                           name=f"vp{b}_{n}")
               for n in range(NT)] for b in range(B)]
        for b in range(B):
            for n in range(NT):
                nc.vector.memset(vp[b][n][:, :, D], 1.0)
                nc.vector.memset(vp[b][n][:, :, 2 * D + 1], 1.0)

        # ---- HAM warmup: real matmul activity spanning the first DMA
        # wait so projections run at 2.4GHz, not the cold 1.2GHz.  The
        # later waves use the weight tiles as rhs so they pace themselves
        # behind the weight DMAs.  (transpose-mode does not warm HAM.)
        # single accumulation group: back-to-back streaming, no per-matmul
        # semaphore round-trips (separate tiles would WAW-serialize)
        wps = psum_pp.tile([128, 128], F32, tag="pp", name="warm")
        for j in range(24):
            nc.tensor.matmul(wps[:], lhsT=ident[:], rhs=ident[:],
                             start=(j == 0), stop=False)
        for kt in range(KT):
            for j in range(3):
                nc.tensor.matmul(wps[:], lhsT=ident[:], rhs=wk_sb[:, kt, :],
                                 start=False,
                                 stop=(kt == KT - 1 and j == 2))

        # ---------- filler units ----------
        # A unit is a list of (pe_cost, closure) ops.  Units are kept in a
        # FIFO; ops are popped a few per st (budget), gated on a DMA-
        # readiness st (gate) and force-drained at the start of the block
        # that consumes their output (need) so a consumer is never emitted
        # before its producer (deadlock-proof).

        def proj_chain_ops(b, name, p, hf):
            """8 matmuls + bias epilogue producing kpT/qpT[b][p] half hf."""
            w_sb, bias, dst = {
                "k": (wk_sb, bk_sb, kpT),
                "q": (wq_sb, bq_sb, qpT),
            }[name]
            box = {}
            ops = []
            for kt in range(KT):
                def mm(kt=kt):
                    if kt == 0:
                        box["ps"] = psum_pp.tile([128, 512], F32, tag="pp",
                                                 name="prch")
                    nc.tensor.matmul(
                        box["ps"][:], lhsT=w_sb[:, kt, :],
                        rhs=staged[name][:, kt,
                                         p * 1024 + hf * 512:
                                         p * 1024 + (hf + 1) * 512],
                        start=(kt == 0), stop=(kt == KT - 1),
                    )
                ops.append((1.0, mm))

            def epi():
                nc.vector.tensor_tensor(
                    dst[b][p][:, hf * 512:(hf + 1) * 512], box["ps"][:],
                    bias[:].to_broadcast((EC, 512)), mybir.AluOpType.add,
                )
            ops.append((0.1, epi))
            return ops

        def vproj_chain_ops(b, p, hf):
            """vp[b][2p+hf] projected DIRECTLY in [key, dim] layout: the
            staged activation is the stationary operand, so out = v.T @ Wv
            per 128-key slice -- no on-chip transpose, no cast ladder.
            One strided copy per slice drops the result into the two
            65-wide head slots (ones column preserved)."""
            n = p * 2 + hf
            ops = []
            for sl in range(STN):
                box = {}
                base = p * 1024 + hf * 512 + sl * 128
                for kt in range(KT):
                    def mm(kt=kt, sl=sl, box=box, base=base):
                        if kt == 0:
                            box["ps"] = psum_pp.tile([128, 128], F32,
                                                     tag="pp", name="vch")
                        nc.tensor.matmul(
                            box["ps"][:],
                            lhsT=staged["v"][:, kt, base:base + 128],
                            rhs=wv_sb[:, kt, :],
                            start=(kt == 0), stop=(kt == KT - 1),
                        )
                    ops.append((0.45, mm))

                def cp(sl=sl, box=box):
                    nc.vector.tensor_copy(
                        vp[b][n][:, sl, :].rearrange(
                            "p (h d) -> p h d", h=2)[:, :, 0:D],
                        box["ps"][:].rearrange("p (h d) -> p h d", h=2),
                    )
                ops.append((0.1, cp))
            return ops

        fillers = []  # FIFO of {gate, need, ops: [(cost, op), ...]}

        def add_unit(gate, need, ops):
            fillers.append({"gate": gate, "need": need, "ops": list(ops)})

        ojq = []  # [(gate, op)] out-projection units, 1 popped per st

        def oj_unit(ot, rowbase, m, pool=None, scalar_cast=False):
            def op():
                # once the filler chains have drained, alternate the ojs
                # across two PSUM banks so the matmul never waits the
                # previous oj's vector cast
                pl = pool
                if pl is None and not fillers and m % 2 == 1:
                    pl = psum_pp
                pt = (pl or psum_oj).tile([128, 512], F32,
                                          tag=("pp" if pl is psum_pp else "oj"),
                                          name="ojp")
                nc.tensor.matmul(pt[:], lhsT=wo_sb[:, m, :], rhs=ot[:],
                                 start=True, stop=True)
                ov = ov_pool.tile([128, 512], BF16, tag="ov", name="ovt")
                if scalar_cast:
                    nc.scalar.copy(ov[:], pt[:])
                else:
                    nc.vector.tensor_copy(ov[:], pt[:])
                nc.sync.dma_start(
                    outTp[m * 128:(m + 1) * 128, rowbase:rowbase + 512], ov[:]
                )
            return op

        def pop_fillers(g, budget=2.4):
            if ojq and g >= ojq[0][0] and g % 2 == 1:
                ojq.pop(0)[1]()
                budget -= 1.0
            spent = 0.0
            while fillers and spent < budget:
                u = fillers[0]
                if g < u["gate"] and u["need"] > g + 8:
                    break
                cost, op = u["ops"].pop(0)
                op()
                spent += cost
                if not u["ops"]:
                    fillers.pop(0)

        def force_units(max_need):
            """Emit every queued unit needed by st <= max_need (and, by
            FIFO, everything ahead of it)."""
            last = -1
            for i, u in enumerate(fillers):
                if u["need"] <= max_need:
                    last = i
            for u in fillers[:last + 1]:
                for _, op in u["ops"]:
                    op()
            del fillers[:last + 1]

        # ---------- core attention ops ----------

        def scores_exp(b, lt, st):
            ps = psum_sc.tile([128, 1024], F32, tag="sc", name="psc")
            for h in range(HC):
                nc.tensor.matmul(
                    ps[:, h * 512:(h + 1) * 512],
                    lhsT=kpT[b][st // 8][h * D:(h + 1) * D,
                                         (st % 8) * 128:(st % 8 + 1) * 128],
                    rhs=qpT[b][lt // 2][h * D:(h + 1) * D,
                                        (lt % 2) * 512:(lt % 2) * 512 + 512],
                    start=True, stop=True,
                    tile_position=(h * D, 0),
                )
            ex = exp_pool.tile([128, 1024], BF16, tag="exp", name="ext")
            nc.scalar.activation(ex[:], ps[:], AF.Exp)
            return ex

        def pv(b, po, st, ex, first, last):
            for h in range(HC):
                nc.tensor.matmul(
                    po[h][0:DP1, :],
                    lhsT=vp[b][st // STN][:, st % STN, h * DP1:(h + 1) * DP1],
                    rhs=ex[:, h * 512:(h + 1) * 512],
                    start=first, stop=last,
                )

        def norm_and_queue_oj(b, lt, po, g_end, tail=False):
            """Copy PSUM->SBUF (frees PV banks fast), normalize, queue the
            out-projection units (gated 3 sts later so the ot multiply has
            landed before the first oj matmul reaches the PE)."""
            otrs, rcs = [], []
            for h in range(HC):
                otr = otr_pool.tile([D, 512], F32, tag="otr", name="otrt")
                sm = rc_pool.tile([1, 512], F32, tag="sm", name="smt")
                if tail and h == 0:
                    # scalar engine is idle after the last exp: split the
                    # tail normalization across engines
                    nc.scalar.copy(otr[:], po[h][0:D, :])
                    nc.scalar.copy(sm[:], po[h][D:DP1, :])
                else:
                    nc.vector.tensor_copy(otr[:], po[h][0:D, :])
                    nc.vector.tensor_copy(sm[:], po[h][D:DP1, :])
                otrs.append(otr)
                rcs.append(sm)
            for h in range(HC):
                rc1 = rc_pool.tile([1, 512], F32, tag="rc1", name="rc1t")
                nc.vector.reciprocal_approx_fast(rc1[:], rcs[h][:])
                rcs[h] = rc1
            rcbs = []
            for h in range(HC):
                rcb = rcb_pool.tile([D, 512], F32, tag="rcb", name="rcbt")
                nc.gpsimd.partition_broadcast(rcb[:], rcs[h][:])
                rcbs.append(rcb)
            ot = ot_pool.tile([128, 512], BF16, tag="ot", name="ott")
            for h in range(HC):
                nc.vector.tensor_tensor(
                    ot[h * D:(h + 1) * D, :], otrs[h][:], rcbs[h][:],
                    mybir.AluOpType.mult,
                )
            rowbase = b * L + lt * 512
            for m in range(KT):
                pool = psum_pp if (tail and m % 2 == 1) else None
                ojq.append((g_end + 3, oj_unit(ot, rowbase, m, pool=pool)))

        # ---------- staging schedule (3 rings, need-order) --------------
        stage_chunk("k", 0, 0, 1)
        stage_chunk("q", 0, 0, 1, eng=nc.scalar)
        stage_chunk("q", 0, 1, 2, eng=nc.scalar)
        dma_plan = {
            0: [("v", 0, 0, 1), ("k", 0, 1, 2)],
            2: [("v", 0, 1, 2), ("k", 0, 2, 4)],
            4: [("v", 0, 2, 4)],
            6: [("q", 0, 2, 4)],
            16: [("k", 1, 0, 2)], 17: [("k", 1, 2, 4)],
            22: [("v", 1, 0, 2)], 24: [("v", 1, 2, 4)],
            42: [("q", 1, 0, 2)], 44: [("q", 1, 2, 4)],
        }

        # ---------- pre-loop projections (block (0,0) first sts) --------
        for _, op in proj_chain_ops(0, "k", 0, 0):
            op()
        for _, op in proj_chain_ops(0, "q", 0, 0):
            op()

        # ---------- filler unit queue, ordered by need (FIFO) -----------
        # gate = earliest st the staging DMA has landed (pop paced);
        # need = first global st whose scores/PV consumes the output
        # (lookahead-forced so a consumer is never emitted first).
        add_unit(23, 32, proj_chain_ops(0, "q", 1, 0))   # blocks (0,2)+
        add_unit(25, 40, proj_chain_ops(0, "q", 1, 1))
        add_unit(28, 64, proj_chain_ops(1, "k", 0, 0))
        add_unit(39, 64, vproj_chain_ops(1, 0, 0))
        add_unit(44, 64, proj_chain_ops(1, "q", 0, 0))
        add_unit(29, 68, proj_chain_ops(1, "k", 0, 1))
        add_unit(40, 68, vproj_chain_ops(1, 0, 1))
        add_unit(34, 72, proj_chain_ops(1, "k", 1, 0))
        add_unit(49, 72, vproj_chain_ops(1, 1, 0))
        add_unit(35, 76, proj_chain_ops(1, "k", 1, 1))
        add_unit(50, 76, vproj_chain_ops(1, 1, 1))
        add_unit(45, 80, proj_chain_ops(1, "q", 0, 1))
        add_unit(54, 96, proj_chain_ops(1, "q", 1, 0))
        add_unit(55, 112, proj_chain_ops(1, "q", 1, 1))

        # ---------- block (0,0): hand-scheduled (DMA-paced) -------------
        # Interleaves the remaining batch-0 projections between the scores
        # (chains sequential: they share the single pp PSUM bank) with PV
        # emission deferred until each vp quarter's transposes are out.
        vch = {n: vproj_chain_ops(0, n // 2, n % 2) for n in range(NT)}
        kch = {1: proj_chain_ops(0, "k", 0, 1),
               2: proj_chain_ops(0, "k", 1, 0),
               3: proj_chain_ops(0, "k", 1, 1)}
        qch1 = proj_chain_ops(0, "q", 0, 1)
        # per-st: (list of (chain, lo, hi) to emit, PV sts to emit after)
        b0_plan = {
            1: ([(kch[1], 0, 5)], []),
            2: ([(kch[1], 5, 9)], []),
            3: ([(vch[0], 0, 18)], []),
            4: ([(vch[0], 18, 36)], [0, 1, 2, 3]),
            5: ([(kch[2], 0, 9)], []),
            6: ([(kch[3], 0, 9)], []),
            7: ([(vch[1], 0, 18)], [4, 5]),
            8: ([(vch[1], 18, 36)], [6, 7]),
            9: ([(vch[2], 0, 18)], []),
            10: ([(vch[2], 18, 36)], [8, 9]),
            11: ([(vch[3], 0, 18)], [10, 11]),
            12: ([(vch[3], 18, 36)], [12]),
            13: ([(qch1, 0, 9)], [13]),
            14: ([], [14]),
            15: ([], [15]),
        }
        po = [psum_pv.tile([128, 512], F32, tag="pv", name=f"po{h}")
              for h in range(HC)]
        exs = {}
        for st in range(ST):
            for ch in dma_plan.get(st, []):
                stage_chunk(*ch)
            exs[st] = scores_exp(0, 0, st)
            pre, pvs = b0_plan.get(st, ([], []))
            for ch, o0, o1 in pre:
                for _, op in ch[o0:o1]:
                    op()
            for p_st in pvs:
                pv(0, po, p_st, exs.pop(p_st), first=(p_st == 0),
                   last=(p_st == ST - 1))
        norm_and_queue_oj(0, 0, po, 15)

        # ---------- blocks 1..7: generic scalar-paced loop --------------
        for blk in range(1, 8):
            b, lt = blk // 4, blk % 4
            po = [psum_pv.tile([128, 512], F32, tag="pv", name=f"po{h}")
                  for h in range(HC)]
            for st in range(ST):
                g = blk * 16 + st
                for ch in dma_plan.get(g, []):
                    stage_chunk(*ch)
                ex = scores_exp(b, lt, st)
                pv(b, po, st, ex, first=(st == 0), last=(st == ST - 1))
                pop_fillers(g)
                force_units(g + 2)
            norm_and_queue_oj(b, lt, po, blk * 16 + 15, tail=(blk == 7))

        # ---------- tail flush ----------
        while fillers:
            u = fillers.pop(0)
            for _, op in u["ops"]:
                op()
        while ojq:
            ojq.pop(0)[1]()

    nc.compile()
    return nc


_NC_CACHE = {}


def _get_nc():
    if "nc" not in _NC_CACHE:
        _NC_CACHE["nc"] = build_nc()
    return _NC_CACHE["nc"]


def _prearrange(w):
    # [E, EC] -> [128, KT*EC] partition-major so the device DMA is contiguous
    bf = ml_dtypes.bfloat16
    return np.ascontiguousarray(
        w.reshape(KT, 128, EC).transpose(1, 0, 2).reshape(128, KT * EC)
    ).astype(bf)


def kernel(q, k, v, Wq, bq, Wk, bk, Wv, bv, Wo, bo, _trace=False, _tmpdir=None):
    bf = ml_dtypes.bfloat16
    scale = np.float32(1.0 / np.sqrt(D))  # 0.125, exact

    def _stage_x(x):
        # [B, L, E] -> [128, B, 2, KT, 1024]: partition-major staging with
        # each L-half contiguous per partition (long DMA lines, few issues)
        xt = np.asarray(x, np.float32).reshape(B, 2, 1024, KT, 128)
        return np.ascontiguousarray(xt.transpose(4, 0, 1, 3, 2)).astype(bf)

    qTh = _stage_x(q)
    kTh = _stage_x(k)
    vTh = _stage_x(v)
    Wq = np.asarray(Wq, np.float32)
    Wk = np.asarray(Wk, np.float32)
    Wv = np.asarray(Wv, np.float32)
    Wo = np.asarray(Wo, np.float32)

    in_maps = []
    for c in range(NCORES):
        sl = slice(c * EC, (c + 1) * EC)
        in_maps.append({
            "qT": qTh,
            "kT": kTh,
            "vT": vTh,
            "wq": _prearrange(Wq[:, sl] * scale),
            "wk": _prearrange(Wk[:, sl]),
            "wv": _prearrange(Wv[:, sl]),
            "wo": np.ascontiguousarray(Wo[sl, :]).astype(bf),
            "bq": (np.asarray(bq, np.float32)[sl] * scale).reshape(EC, 1).copy(),
            "bk": np.asarray(bk, np.float32)[sl].reshape(EC, 1).copy(),
        })

    nc = _get_nc()
    res = run_bass_kernel_spmd(
        nc, in_maps, list(range(NCORES)), trace=_trace, tmpdir=_tmpdir
    )
    # sum the per-core partial outputs (the all-reduce of the TP sharding)
    acc = np.zeros((E, R), np.float32)
    for c in range(NCORES):
        acc += np.asarray(res.results[c]["outTp"], np.float32)
    out = np.ascontiguousarray(acc.T)  # [R, E]
    # bv passes through attention unchanged (softmax rows sum to 1):
    # out += bv @ Wo + bo
    host_bias = (
        np.asarray(bv, np.float64) @ np.asarray(Wo, np.float64)
        + np.asarray(bo, np.float64)
    ).astype(np.float32)
    out += host_bias[None, :]
    if _trace:
        return out.reshape(B, L, E), res
    return out.reshape(B, L, E)
